# revision 11
# baseline (speedup 1.0000x reference)
"""NCD-via-LZW kernel for Trainium2 (8 NeuronCores, Bass).

Problem: quantize x [32,3,32,32] to 8 levels along a space-filling curve =>
96 strings of length 1024; LZW-compress the 96 strings, the 48 pattern maps,
and the 1536 string||pmap concatenations; return the normalized compression
distance matrix [32, 48].

Mapping: LZW is sequential per sequence but there are 1680 independent
sequences. Each NeuronCore runs 2 "waves" of 128 lanes (one LZW sequence per
SBUF partition), ~2048 sequential steps per wave, 4 stock DVE instructions
per step. Sharding: core n handles batches 4n..4n+3 (192 concat runs + 12
string runs) plus 6 of the 48 pmap runs.

Per-lane LZW state (all exact in fp32):
  key(cur, c) = cur + (c+1)/16  (cur = integer trie node id, c in [0,8))
  EK[t] = key inserted at step t on miss, 0.0 on hit (queries >= 1/16 > 0)
  EN[t] = 7 + (#misses after step t)  (= node id created at the last miss)
Step t (query in keyp1, match result in acc):
  1. acc = sum_j (EK[0:t] == keyp1) * EN[0:t]   -- scalar_tensor_tensor with
     fused accum; = matched node id or 0 (at most one slot matches)
  2. EN[t] = (acc == 0) + EN[t-1]
  3. EK[t] = (acc == 0) * keyp1
  4. keyp1 = max(acc, c_t) + (c_{t+1}+1)/16     (node ids >= 8 > c_t)
lzw_count(seq[0:L]) = EN[L-1] - 6.
"""

import numpy as np

B, C, H, W = 32, 3, 32, 32
L = 8
P = 16
M = 1024
N = H * W
T = 2048
NCORES = 8
OUT_COLS = (1023, 2047)

_nc_cache = {}


class _Chain:
    """Same-engine serialization via an attached-wait semaphore chain (the
    pattern Tile emits for same-engine RAW deps). With sem=None, relies on
    the engine's in-order execution + per-op pipeline drain instead (each
    DVE op's DRAIN is the output-dependency barrier on HW)."""

    def __init__(self, sem):
        self.sem = sem
        self.k = 0

    def add(self, inst):
        if self.sem is not None:
            inst._wait_ge(self.sem, self.k)
            inst.then_inc(self.sem)
        self.k += 1
        return inst


def _emit_wave(vector, ch, S, EK, EN, scratch, acc, keyp1, T_):
    import concourse.mybir as mybir
    AO = mybir.AluOpType

    ch.add(vector.memset(acc[:], 0.0))
    ch.add(vector.memset(EK[:, 0:1], 0.0))
    ch.add(vector.memset(EN[:, 0:1], 7.0))
    ch.add(vector.scalar_tensor_tensor(
        keyp1[:], acc[:], S[:, 0:1], S[:, 1:2], AO.max, AO.add))
    for t in range(1, T_):
        if t >= 2:
            ch.add(vector.scalar_tensor_tensor(
                scratch[:, 0:t], EK[:, 0:t], keyp1[:], EN[:, 0:t],
                AO.is_equal, AO.mult, accum_out=acc[:]))
        ch.add(vector.tensor_scalar(
            EN[:, t:t + 1], acc[:], 0.0, EN[:, t - 1:t],
            AO.is_equal, AO.add))
        ch.add(vector.scalar_tensor_tensor(
            EK[:, t:t + 1], acc[:], 0.0, keyp1[:], AO.is_equal, AO.mult))
        ch.add(vector.scalar_tensor_tensor(
            keyp1[:], acc[:], S[:, 2 * t:2 * t + 1],
            S[:, 2 * t + 1:2 * t + 2], AO.max, AO.add))


def _emit_wave_i16(vector, ch, S, EK, EN, scratch, acc, keyp1, g, T_):
    """int16 entry arrays: key16 = cur*16 + c + 1 (<= 16616, int16-exact).
    Entry match runs in the DVE 2x_1P perf mode (2-byte dtype, step 1).
    S cols: 2t = c_t, 2t+1 = c_{t+1}+1."""
    import concourse.mybir as mybir
    AO = mybir.AluOpType

    ch.add(vector.memset(acc[:], 0.0))
    ch.add(vector.memset(EK[:, 0:1], 0))
    ch.add(vector.memset(EN[:, 0:1], 7))
    # keyp1_1 = max(0, c_0)*16 + (c_1+1)
    ch.add(vector.tensor_scalar(
        g[:], acc[:], S[:, 0:1], 16.0, AO.max, AO.mult))
    ch.add(vector.tensor_scalar(
        keyp1[:], g[:], S[:, 1:2], None, AO.add))
    for t in range(1, T_):
        if t >= 2:
            ch.add(vector.scalar_tensor_tensor(
                scratch[:, 0:t], EK[:, 0:t], keyp1[:], EN[:, 0:t],
                AO.is_equal, AO.mult, accum_out=acc[:]))
        ch.add(vector.scalar_tensor_tensor(
            EN[:, t:t + 1], acc[:], 0.0, EN[:, t - 1:t],
            AO.is_equal, AO.add))
        ch.add(vector.scalar_tensor_tensor(
            EK[:, t:t + 1], acc[:], 0.0, keyp1[:], AO.is_equal, AO.mult))
        ch.add(vector.tensor_scalar(
            g[:], acc[:], S[:, 2 * t:2 * t + 1], 16.0, AO.max, AO.mult))
        ch.add(vector.tensor_scalar(
            keyp1[:], g[:], S[:, 2 * t + 1:2 * t + 2], None, AO.add))


def _build_program(use_chain=True, i16=False):
    import concourse.bass as bass
    import concourse.mybir as mybir

    key = ("nc", use_chain, i16)
    if key in _nc_cache:
        return _nc_cache[key]

    dt = mybir.dt.float32
    edt = mybir.dt.int16 if i16 else dt
    nc = bass.Bass()
    n_waves, nocc = 2, len(OUT_COLS)

    syms_d = [nc.declare_dram_parameter(f"syms{w}", [128, 2 * T], dt,
                                        isOutput=False) for w in range(n_waves)]
    out_d = nc.declare_dram_parameter("counts", [128, n_waves * nocc], dt,
                                      isOutput=True)

    sym = [nc.alloc_sbuf_tensor(f"sym{w}", [128, 2 * T], dt).ap()
           for w in range(n_waves)]
    EK = nc.alloc_sbuf_tensor("EK", [128, T], edt).ap()
    EN = nc.alloc_sbuf_tensor("EN", [128, T], edt).ap()
    scratch = nc.alloc_sbuf_tensor("scratch", [128, T], edt).ap()
    acc = nc.alloc_sbuf_tensor("acc", [128, 1], dt).ap()
    keyp1 = nc.alloc_sbuf_tensor("keyp1", [128, 1], dt).ap()
    g = nc.alloc_sbuf_tensor("g", [128, 1], dt).ap()
    outt = nc.alloc_sbuf_tensor("outt", [128, n_waves * nocc], dt).ap()

    dma_sem = nc.alloc_semaphore("dma_sem")
    chain_sem = nc.alloc_semaphore("chain_sem")
    done_sem = nc.alloc_semaphore("done_sem")

    per_wave = (5 + 5 * (T - 2) + 4) if i16 else (4 + 3 + 4 * (T - 2))
    total_chain = n_waves * (per_wave + nocc)

    with nc.Block() as block:

        @block.sync
        def _(sync):
            for w in range(n_waves):
                sync.dma_start(sym[w][:], syms_d[w][:]).then_inc(dma_sem, 16)
            if use_chain:
                sync.wait_ge(chain_sem, total_chain)
            else:
                sync.wait_ge(done_sem, 1)
            sync.dma_start(out_d[:], outt[:]).then_inc(dma_sem, 16)

        @block.vector
        def _(vector):
            vector.wait_ge(dma_sem, 16 * n_waves)
            ch = _Chain(chain_sem if use_chain else None)
            for w in range(n_waves):
                if i16:
                    _emit_wave_i16(vector, ch, sym[w], EK, EN, scratch, acc,
                                   keyp1, g, T)
                else:
                    _emit_wave(vector, ch, sym[w], EK, EN, scratch, acc,
                               keyp1, T)
                for i, col in enumerate(OUT_COLS):
                    last = ch.add(vector.tensor_copy(
                        outt[:, w * nocc + i:w * nocc + i + 1],
                        EN[:, col:col + 1]))
            assert ch.k == total_chain, (ch.k, total_chain)
            if not use_chain:
                last.then_inc(done_sem)

    _nc_cache[key] = nc
    return nc


_I16 = False  # kernel variant selector


def _prestage(syms, i16=None):
    """[n, T] int symbols -> [n, 2T] f32: col 2t = c_t,
    col 2t+1 = (c_{t+1}+1)/16 (fp32 variant) or c_{t+1}+1 (i16 variant)."""
    if i16 is None:
        i16 = _I16
    syms = np.asarray(syms, np.float32)
    n, T_ = syms.shape
    out = np.zeros((n, 2 * T_), np.float32)
    out[:, ::2] = syms
    nxt = syms[:, 1:] + 1.0
    out[:, 1:2 * T_ - 2:2] = nxt if i16 else nxt / 16.0
    return out


def _quantize(x, curve, levels):
    """x [B,C,H,W] -> strings [B,C,N] int32 (nearest level, first-min)."""
    out = np.asarray(x, np.float32).reshape(B, C, -1)[:, :, np.asarray(curve)]
    lv = np.asarray(levels, np.float32)
    return np.argmin(
        np.abs(out[:, :, None, :] - lv[:, None].reshape(1, C, L, 1)), axis=2
    ).astype(np.int32)


def _lane_symbols(strings, pmaps):
    """Build per-core wave symbol matrices.

    Returns (syms0, syms1): each [8][128, 2T] f32 prestaged.
    Core n: wave0 lanes 0..127 and wave1 lanes 0..63 hold concat runs
    j = (b_loc*3 + c)*16 + k (b = 4n + b_loc); wave1 lanes 64..75 hold the 12
    string runs (b_loc*3 + c); lanes 76..81 hold pmap runs cp = 6n..6n+5."""
    pm = np.asarray(pmaps, np.int64)
    syms0, syms1 = [], []
    for n in range(NCORES):
        w0 = np.zeros((128, T), np.int64)
        w1 = np.zeros((128, T), np.int64)
        for j in range(192):
            b_loc, c, k = j // 48, (j // 16) % 3, j % 16
            row = np.concatenate([strings[4 * n + b_loc, c], pm[c, k]])
            if j < 128:
                w0[j] = row
            else:
                w1[j - 128] = row
        for idx in range(12):
            b_loc, c = idx // 3, idx % 3
            w1[64 + idx, :N] = strings[4 * n + b_loc, c]
        for jj in range(6):
            cp = 6 * n + jj
            w1[76 + jj, :M] = pm[cp // 16, cp % 16]
        syms0.append(_prestage(w0))
        syms1.append(_prestage(w1))
    return syms0, syms1


def _assemble(results):
    """results[n]['counts'] [128, 4] -> ncd [32, 48] f32."""
    c_s = np.zeros((B, C), np.float32)
    c_p = np.zeros((C, P), np.float32)
    c_sp = np.zeros((B, C, P), np.float32)
    for n in range(NCORES):
        cnts = np.asarray(results[n]["counts"], np.float32) - 6.0
        for j in range(192):
            b_loc, c, k = j // 48, (j // 16) % 3, j % 16
            col = 1 if j < 128 else 3
            lane = j if j < 128 else j - 128
            c_sp[4 * n + b_loc, c, k] = cnts[lane, col]
        for idx in range(12):
            b_loc, c = idx // 3, idx % 3
            c_s[4 * n + b_loc, c] = cnts[64 + idx, 2]
        for jj in range(6):
            cp = 6 * n + jj
            c_p[cp // 16, cp % 16] = cnts[76 + jj, 2]
    ncd = (c_sp - np.minimum(c_s[:, :, None], c_p[None, :, :])) / np.maximum(
        c_s[:, :, None], c_p[None, :, :])
    return ncd.reshape(B, C * P).astype(np.float32)


def _run(in_maps, trace=False):
    from concourse.bass_utils import run_bass_kernel_spmd
    nc = _build_program(i16=_I16)
    return run_bass_kernel_spmd(nc, in_maps, list(range(NCORES)), trace=trace)


def _in_maps(x, curve, levels, pmaps):
    strings = _quantize(x, curve, levels)
    syms0, syms1 = _lane_symbols(strings, pmaps)
    return [{"syms0": syms0[n], "syms1": syms1[n]} for n in range(NCORES)]


def kernel(x, curve, levels, pmaps, i=0, **_unused):
    del i
    in_maps = _in_maps(x, curve, levels, pmaps)
    res = _run(in_maps)
    return _assemble([res.results[n] for n in range(NCORES)])


def kernel_profiled(x, curve, levels, pmaps, i=0, **_unused):
    """Like kernel() but with NTFF tracing; returns (out, exec_time_ns).
    Falls back to (out, None) when the profiling hook is unavailable."""
    del i
    in_maps = _in_maps(x, curve, levels, pmaps)
    try:
        res = _run(in_maps, trace=True)
        return (_assemble([res.results[n] for n in range(NCORES)]),
                res.exec_time_ns)
    except Exception:
        res = _run(in_maps)
        return _assemble([res.results[n] for n in range(NCORES)]), None
